# revision 21
# baseline (speedup 1.0000x reference)
"""Trainium2 Bass kernel for the DCN cross layer.

Computes out = x0 * (x_cross @ w)[:, None] + b + x_cross for
x0, x_cross: [16384, 4096] f32, w, b: [4096] f32.

Sharding: pure data parallel — batch split across 8 NeuronCores,
w and b replicated. Each core processes a [2048, 4096] shard.

The op is memory-bound (3 HBM streams, no reuse) and the f32 version
sits at the 358 GB/s/core DMA roofline, so all I/O is done in bf16:
the host casts inputs once, the device computes in bf16 with an f32
dot-product accumulator, and the host upcasts the output. Error is
~0.1% in norm, well under the 2e-2 gate.
"""

import sys

import numpy as np

sys.path.insert(0, "/opt/trn_rl_repo")

import ml_dtypes

BF16 = ml_dtypes.bfloat16

N_CORES = 8
BATCH = 16384
D = 4096
ROWS_PER_CORE = BATCH // N_CORES  # 2048
P = 128
RPP = 1  # rows per partition per tile -> DMA transfer size = RPP * 1 MB
BUFS = 4

_NC = None


def _build(rpp=None, bufs=None, tmp_bufs=3, s_bufs=4, reduce_mode="stt"):
    """Build + schedule the single-core SPMD program (same on all cores).

    Engine facts measured on HW: DVE tensor_tensor runs at 2x (2.29us per
    [128,4096] bf16 tile), but every DVE op with a free-dim reduction
    (scalar_tensor_tensor, TENSOR_SCALAR_CACHE_REDUCE) runs at 1x
    (~4.4us). ACT activation is 3.8us/tile and also has an accum_out
    rowsum. The Pool engine is useless here (TensorScalarPtr fails its ISA
    check; its tensor_tensor measured ~2x slower than the model).

    reduce_mode='stt':  DVE stt junk=xc*w + accum s (4.4us)
    reduce_mode='act':  DVE tt junk=xc*w (2.3us), ACT Copy(junk)+accum s
                        (3.8us) - moves the reduce cost to the ACT engine

    Then t = x0*s on ACT (per-partition scale AP), u = t+xc and out = u+b
    on DVE at 2x. The final adds for tile i are emitted one iteration
    late (software pipelining): each engine's queue executes in program
    order, so without this the DVE sits in u_i waiting on ACT's t_i while
    the ready stt_{i+1} is stuck behind it in the queue.
    """
    from contextlib import ExitStack

    import concourse.tile as tile
    from concourse import bacc, mybir

    rpp = RPP if rpp is None else rpp
    bufs = BUFS if bufs is None else bufs

    bf16 = mybir.dt.bfloat16
    f32 = mybir.dt.float32
    mult = mybir.AluOpType.mult
    add = mybir.AluOpType.add

    nc = bacc.Bacc(
        "TRN2", target_bir_lowering=False, debug=False, num_devices=N_CORES
    )
    x0_d = nc.dram_tensor("x0", [ROWS_PER_CORE, D], bf16, kind="ExternalInput").ap()
    xc_d = nc.dram_tensor(
        "x_cross", [ROWS_PER_CORE, D], bf16, kind="ExternalInput"
    ).ap()
    w_d = nc.dram_tensor("w", [D], bf16, kind="ExternalInput").ap()
    b_d = nc.dram_tensor("b", [D], bf16, kind="ExternalInput").ap()
    out_d = nc.dram_tensor(
        "out", [ROWS_PER_CORE, D], bf16, kind="ExternalOutput"
    ).ap()

    rows_per_tile = P * rpp
    n_tiles = ROWS_PER_CORE // rows_per_tile
    with tile.TileContext(nc) as tc, ExitStack() as ctx:
        consts = ctx.enter_context(tc.tile_pool(name="consts", bufs=1))
        xc_pool = ctx.enter_context(tc.tile_pool(name="xc", bufs=bufs))
        x0_pool = ctx.enter_context(tc.tile_pool(name="x0", bufs=bufs))
        junk_pool = ctx.enter_context(tc.tile_pool(name="junk", bufs=2))
        t_pool = ctx.enter_context(tc.tile_pool(name="t", bufs=tmp_bufs))
        u_pool = ctx.enter_context(tc.tile_pool(name="u", bufs=tmp_bufs))
        s_pool = ctx.enter_context(tc.tile_pool(name="s", bufs=s_bufs))

        # w and b replicated across all 128 partitions (one-time). Load the
        # 8 KB rows once from HBM, then broadcast SBUF->SBUF on the idle PE
        # ring: the old DRAM partition_broadcast re-read 2 MiB of HBM and
        # held up the first stt by ~11 us.
        w_row = consts.tile([1, D], bf16)
        b_row = consts.tile([1, D], bf16)
        nc.sync.dma_start(out=w_row[:], in_=w_d.partition_broadcast(1))
        nc.sync.dma_start(out=b_row[:], in_=b_d.partition_broadcast(1))
        w_t = consts.tile([P, D], bf16)
        b_t = consts.tile([P, D], bf16)
        nc.gpsimd.partition_broadcast(w_t[:], w_row[0:1, :])
        nc.gpsimd.partition_broadcast(b_t[:], b_row[0:1, :])

        assert rpp == 1, "software-pipelined loop assumes rpp == 1"

        def finish(prev):
            """Emit tile i's ACT-dependent tail (u, out, store)."""
            xc_p, x0_p, t_p, r0_p, splits = prev
            u_t = u_pool.tile([P, D], bf16)
            cn = D // splits
            for k in range(splits):
                dsk = slice(k * cn, (k + 1) * cn)
                # u = t + xc  (2x mode)
                nc.vector.tensor_add(u_t[:, dsk], t_p[:, dsk], xc_p[:, dsk])
                # out = u + b  (2x mode); x0 is dead, reuse it as the output
                nc.vector.tensor_add(x0_p[:, dsk], u_t[:, dsk], b_t[:, dsk])
                # store from the ACT HWDGE ring so loads (SP ring) and
                # stores use separate descriptor generators
                nc.scalar.dma_start(
                    out=out_d[
                        r0_p : r0_p + rows_per_tile, k * cn : (k + 1) * cn
                    ].rearrange("(p r) d -> p (r d)", p=P),
                    in_=x0_p[:, dsk],
                )

        prev = None
        for i in range(n_tiles):
            r0 = i * rows_per_tile
            # [rows_per_tile, D] DRAM block == [P, D] SBUF tile
            xc_t = xc_pool.tile([P, D], bf16)
            nc.sync.dma_start(
                out=xc_t[:],
                in_=xc_d[r0 : r0 + rows_per_tile, :].rearrange(
                    "(p r) d -> p (r d)", p=P
                ),
            )
            x0_t = x0_pool.tile([P, D], bf16)
            nc.sync.dma_start(
                out=x0_t[:],
                in_=x0_d[r0 : r0 + rows_per_tile, :].rearrange(
                    "(p r) d -> p (r d)", p=P
                ),
            )

            s_t = s_pool.tile([P, 1], f32)
            if reduce_mode == "stt":
                # junk = xc * w (discarded), s = rowsum(xc * w), 1x rate
                junk_t = junk_pool.tile([P, D], bf16)
                nc.vector.scalar_tensor_tensor(
                    out=junk_t[:],
                    in0=xc_t[:],
                    scalar=1.0,
                    in1=w_t[:],
                    op0=mult,
                    op1=mult,
                    accum_out=s_t[:],
                )
            else:
                # junk = xc * w on DVE (2x), rowsum on ACT via Copy+accum
                junk_t = junk_pool.tile([P, D], bf16)
                nc.vector.tensor_tensor(junk_t[:], xc_t[:], w_t[:], mult)
                junk2_t = junk_pool.tile([P, D], bf16)
                nc.scalar.activation(
                    out=junk2_t[:],
                    in_=junk_t[:],
                    func=mybir.ActivationFunctionType.Copy,
                    accum_out=s_t[:],
                )
            # t = x0 * s on the ACT engine (activation Copy with a
            # per-partition scale AP). The last tile's tail is the exit
            # critical path (nothing left to overlap with), so split its
            # post-s ops in half to shorten the drain.
            splits = 2 if i == n_tiles - 1 else 1
            t_t = t_pool.tile([P, D], bf16)
            cn = D // splits
            for k in range(splits):
                dsk = slice(k * cn, (k + 1) * cn)
                nc.scalar.mul(t_t[:, dsk], x0_t[:, dsk], s_t[:])

            if prev is not None:
                finish(prev)
            prev = (xc_t, x0_t, t_t, r0, splits)
        finish(prev)

    nc.compile()
    return nc


def _get_nc():
    global _NC
    if _NC is None:
        _NC = _build()
    return _NC


def _run(inputs, trace=False, **spmd_kwargs):
    """Shard, run on 8 cores, gather. Returns (full_output, BassKernelResults)."""
    from concourse.bass_utils import run_bass_kernel_spmd

    nc = _get_nc()

    x0 = np.ascontiguousarray(np.asarray(inputs["x0"]).astype(BF16))
    xc = np.ascontiguousarray(np.asarray(inputs["x_cross"]).astype(BF16))
    w = np.ascontiguousarray(np.asarray(inputs["w"]).astype(BF16))
    b = np.ascontiguousarray(np.asarray(inputs["b"]).astype(BF16))

    in_maps = [
        {
            "x0": x0[i * ROWS_PER_CORE : (i + 1) * ROWS_PER_CORE],
            "x_cross": xc[i * ROWS_PER_CORE : (i + 1) * ROWS_PER_CORE],
            "w": w,
            "b": b,
        }
        for i in range(N_CORES)
    ]

    res = run_bass_kernel_spmd(
        nc, in_maps, core_ids=list(range(N_CORES)), trace=trace, **spmd_kwargs
    )
    out = np.concatenate(
        [res.results[i]["out"] for i in range(N_CORES)], axis=0
    ).astype(np.float32)
    return out, res


def kernel(**inputs: np.ndarray) -> np.ndarray:
    out, _ = _run(inputs)
    return out


# revision 24
# speedup vs baseline: 1.0467x; 1.0467x over previous
"""Trainium2 Bass kernel for the DCN cross layer.

Computes out = x0 * (x_cross @ w)[:, None] + b + x_cross for
x0, x_cross: [16384, 4096] f32, w, b: [4096] f32.

Sharding: pure data parallel — batch split across 8 NeuronCores,
w and b replicated. Each core processes a [2048, 4096] shard.

The op is memory-bound (3 HBM streams, no reuse) and the f32 version
sits at the 358 GB/s/core DMA roofline, so all I/O is done in bf16:
the host casts inputs once, the device computes in bf16 with an f32
dot-product accumulator, and the host upcasts the output. Error is
~0.1% in norm, well under the 2e-2 gate.
"""

import sys

import numpy as np

sys.path.insert(0, "/opt/trn_rl_repo")

import ml_dtypes

BF16 = ml_dtypes.bfloat16

N_CORES = 8
BATCH = 16384
D = 4096
ROWS_PER_CORE = BATCH // N_CORES  # 2048
P = 128
RPP = 1  # rows per partition per tile -> DMA transfer size = RPP * 1 MB
BUFS = 4

_NC = None


def _build(rpp=None, bufs=None, tmp_bufs=3, s_bufs=4, reduce_mode="stt"):
    """Build + schedule the single-core SPMD program (same on all cores).

    Engine facts measured on HW: DVE tensor_tensor runs at 2x (2.29us per
    [128,4096] bf16 tile), but every DVE op with a free-dim reduction
    (scalar_tensor_tensor, TENSOR_SCALAR_CACHE_REDUCE) runs at 1x
    (~4.4us). ACT activation is 3.8us/tile and also has an accum_out
    rowsum. The Pool engine is useless here (TensorScalarPtr fails its ISA
    check; its tensor_tensor measured ~2x slower than the model).

    reduce_mode='stt':  DVE stt junk=xc*w + accum s (4.4us)
    reduce_mode='act':  DVE tt junk=xc*w (2.3us), ACT Copy(junk)+accum s
                        (3.8us) - moves the reduce cost to the ACT engine

    Then t = x0*s on ACT (per-partition scale AP), u = t+xc and out = u+b
    on DVE at 2x. The final adds for tile i are emitted one iteration
    late (software pipelining): each engine's queue executes in program
    order, so without this the DVE sits in u_i waiting on ACT's t_i while
    the ready stt_{i+1} is stuck behind it in the queue.
    """
    from contextlib import ExitStack

    import concourse.tile as tile
    from concourse import bacc, mybir

    rpp = RPP if rpp is None else rpp
    bufs = BUFS if bufs is None else bufs

    bf16 = mybir.dt.bfloat16
    f32 = mybir.dt.float32
    mult = mybir.AluOpType.mult
    add = mybir.AluOpType.add

    nc = bacc.Bacc(
        "TRN2", target_bir_lowering=False, debug=False, num_devices=N_CORES
    )
    x0_d = nc.dram_tensor("x0", [ROWS_PER_CORE, D], bf16, kind="ExternalInput").ap()
    xc_d = nc.dram_tensor(
        "x_cross", [ROWS_PER_CORE, D], bf16, kind="ExternalInput"
    ).ap()
    # w and b are staged host-replicated as [P, D] so the device gets them
    # with one plain linear DMA each (a stride-0 DRAM broadcast stalled the
    # first reduce ~11us; gpsimd's partition_broadcast was ~9us slower).
    w_d = nc.dram_tensor("w", [P, D], bf16, kind="ExternalInput").ap()
    b_d = nc.dram_tensor("b", [P, D], bf16, kind="ExternalInput").ap()
    out_d = nc.dram_tensor(
        "out", [ROWS_PER_CORE, D], bf16, kind="ExternalOutput"
    ).ap()

    rows_per_tile = P * rpp
    n_tiles = ROWS_PER_CORE // rows_per_tile
    with tile.TileContext(nc) as tc, ExitStack() as ctx:
        consts = ctx.enter_context(tc.tile_pool(name="consts", bufs=1))
        xc_pool = ctx.enter_context(tc.tile_pool(name="xc", bufs=bufs))
        x0_pool = ctx.enter_context(tc.tile_pool(name="x0", bufs=bufs))
        junk_pool = ctx.enter_context(tc.tile_pool(name="junk", bufs=2))
        t_pool = ctx.enter_context(tc.tile_pool(name="t", bufs=tmp_bufs))
        u_pool = ctx.enter_context(tc.tile_pool(name="u", bufs=tmp_bufs))
        s_pool = ctx.enter_context(tc.tile_pool(name="s", bufs=s_bufs))

        # one-time consts: plain linear 1 MiB DMAs on the otherwise-idle
        # gpsimd ring, in parallel with the SP-ring x0/xc stream
        w_t = consts.tile([P, D], bf16)
        b_t = consts.tile([P, D], bf16)
        nc.gpsimd.dma_start(out=w_t[:], in_=w_d[:, :])
        nc.gpsimd.dma_start(out=b_t[:], in_=b_d[:, :])

        assert rpp == 1, "software-pipelined loop assumes rpp == 1"

        def finish(prev):
            """Emit tile i's ACT-dependent tail (u, out, store)."""
            xc_p, x0_p, t_p, r0_p, splits = prev
            u_t = u_pool.tile([P, D], bf16)
            cn = D // splits
            for k in range(splits):
                dsk = slice(k * cn, (k + 1) * cn)
                # u = t + xc  (2x mode)
                nc.vector.tensor_add(u_t[:, dsk], t_p[:, dsk], xc_p[:, dsk])
                # out = u + b  (2x mode); x0 is dead, reuse it as the output
                nc.vector.tensor_add(x0_p[:, dsk], u_t[:, dsk], b_t[:, dsk])
                # store from the ACT HWDGE ring so loads (SP ring) and
                # stores use separate descriptor generators
                nc.scalar.dma_start(
                    out=out_d[
                        r0_p : r0_p + rows_per_tile, k * cn : (k + 1) * cn
                    ].rearrange("(p r) d -> p (r d)", p=P),
                    in_=x0_p[:, dsk],
                )

        prev = None
        for i in range(n_tiles):
            r0 = i * rows_per_tile
            # [rows_per_tile, D] DRAM block == [P, D] SBUF tile
            xc_t = xc_pool.tile([P, D], bf16)
            nc.sync.dma_start(
                out=xc_t[:],
                in_=xc_d[r0 : r0 + rows_per_tile, :].rearrange(
                    "(p r) d -> p (r d)", p=P
                ),
            )
            x0_t = x0_pool.tile([P, D], bf16)
            nc.sync.dma_start(
                out=x0_t[:],
                in_=x0_d[r0 : r0 + rows_per_tile, :].rearrange(
                    "(p r) d -> p (r d)", p=P
                ),
            )

            s_t = s_pool.tile([P, 1], f32)
            if reduce_mode == "stt":
                # junk = xc * w (discarded), s = rowsum(xc * w), 1x rate
                junk_t = junk_pool.tile([P, D], bf16)
                nc.vector.scalar_tensor_tensor(
                    out=junk_t[:],
                    in0=xc_t[:],
                    scalar=1.0,
                    in1=w_t[:],
                    op0=mult,
                    op1=mult,
                    accum_out=s_t[:],
                )
            else:
                # junk = xc * w on DVE (2x), rowsum on ACT via Copy+accum
                junk_t = junk_pool.tile([P, D], bf16)
                nc.vector.tensor_tensor(junk_t[:], xc_t[:], w_t[:], mult)
                junk2_t = junk_pool.tile([P, D], bf16)
                nc.scalar.activation(
                    out=junk2_t[:],
                    in_=junk_t[:],
                    func=mybir.ActivationFunctionType.Copy,
                    accum_out=s_t[:],
                )
            # t = x0 * s on the ACT engine (activation Copy with a
            # per-partition scale AP). The last tile's tail is the exit
            # critical path (nothing left to overlap with), so split its
            # post-s ops in half to shorten the drain.
            splits = 2 if i == n_tiles - 1 else 1
            t_t = t_pool.tile([P, D], bf16)
            cn = D // splits
            for k in range(splits):
                dsk = slice(k * cn, (k + 1) * cn)
                nc.scalar.mul(t_t[:, dsk], x0_t[:, dsk], s_t[:])

            if prev is not None:
                finish(prev)
            prev = (xc_t, x0_t, t_t, r0, splits)
        finish(prev)

    nc.compile()
    return nc


def _get_nc():
    global _NC
    if _NC is None:
        _NC = _build()
    return _NC


def _run(inputs, trace=False, **spmd_kwargs):
    """Shard, run on 8 cores, gather. Returns (full_output, BassKernelResults)."""
    from concourse.bass_utils import run_bass_kernel_spmd

    nc = _get_nc()

    x0 = np.ascontiguousarray(np.asarray(inputs["x0"]).astype(BF16))
    xc = np.ascontiguousarray(np.asarray(inputs["x_cross"]).astype(BF16))
    w = np.ascontiguousarray(
        np.broadcast_to(np.asarray(inputs["w"]).astype(BF16), (P, D))
    )
    b = np.ascontiguousarray(
        np.broadcast_to(np.asarray(inputs["b"]).astype(BF16), (P, D))
    )

    in_maps = [
        {
            "x0": x0[i * ROWS_PER_CORE : (i + 1) * ROWS_PER_CORE],
            "x_cross": xc[i * ROWS_PER_CORE : (i + 1) * ROWS_PER_CORE],
            "w": w,
            "b": b,
        }
        for i in range(N_CORES)
    ]

    res = run_bass_kernel_spmd(
        nc, in_maps, core_ids=list(range(N_CORES)), trace=trace, **spmd_kwargs
    )
    out = np.concatenate(
        [res.results[i]["out"] for i in range(N_CORES)], axis=0
    ).astype(np.float32)
    return out, res


def kernel(**inputs: np.ndarray) -> np.ndarray:
    out, _ = _run(inputs)
    return out
